# revision 11
# baseline (speedup 1.0000x reference)
"""Trainium2 Bass kernel for nn_MemoryManager (ToMe token merging).

Problem: x [8, 4096, 4096] fp32, target 1024 tokens. The reference performs 2
bipartite merge steps (4096->2048->1024). Because r == p/2 at both steps, the
"unmerged" set is empty and the argsort is dead code; the weighted sums
telescope so the final output is a single scatter-add of all 4096 original
rows into 1024 accumulators (divided by multiplicity), with all control logic
(metric means, cosine scores, argmax, index composition) in 128-dim metric
space.

Sharding: data-parallel over batch (8 cores, 1 batch element each).

Per-core pipeline:
  A) stream x tiles (even/odd token split), head-sum via tree adds -> mx
     [tok,d]; PE-transpose -> mxT [d,tok].
  B) normalize odd-side metric columns (ones-matmul norms + exact reciprocal
     of sqrt + 2 Newton steps); scores1 = mxT_even^T @ mxT_odd_n in fp32;
     hardware argmax (max/max_index) -> idx1.
  C) one-hot(idx1) matmuls accumulate macc^T = mxT_odd_raw + scatter(mx_even);
     normalize odd columns; scores2 -> idx2.
  D) compose final assignment F (token -> output row) via g-table reduce.
  E) scatter: one-hot(F) matmuls in bf16 (one-hot is exact; x cast to bf16
     during DMA) accumulate out = S_F^T @ x per (dst-block, c-block) in PSUM;
     counts via ones^T @ S_F; out = acc * (1/s2).
"""

import numpy as np

import concourse.bacc as bacc
import concourse.bass as bass
import concourse.mybir as mybir
import concourse.tile as tile
from concourse.bass_utils import run_bass_kernel_spmd
from concourse.masks import make_identity

F32 = mybir.dt.float32
BF16 = mybir.dt.bfloat16
U32 = mybir.dt.uint32

P = 128           # partitions
NT = 16           # token tiles per parity (16 * 128 even + 16 * 128 odd = 4096)
C = 4096          # channels
D = 128           # metric dim (C / 32 heads)
PTOK = 4096       # tokens
N1 = 2048         # step-1 odd/even count
N2 = 1024         # step-2 odd/even count = output rows
AL = mybir.AluOpType

_LVL = {"A": 0, "B": 1, "C": 2, "D": 3, "E": 4}


def _newton_rsqrt(nc, pool, y, n2, shape):
    """y <- rsqrt(n2) refined: two Newton steps y *= 1.5 - 0.5*n2*y*y."""
    t = pool.tile(shape, F32, tag="newt_t", name="newt_t")
    for _ in range(2):
        nc.vector.tensor_mul(t[:], y, y)
        nc.vector.tensor_mul(t[:], t[:], n2)
        nc.vector.tensor_scalar(t[:], t[:], -0.5, 1.5, AL.mult, AL.add)
        nc.vector.tensor_mul(y, y, t[:])


def build_kernel(stop_after="E"):
    lvl = _LVL[stop_after]
    nc = bacc.Bacc(None, target_bir_lowering=False)
    x = nc.dram_tensor("x", [PTOK, C], F32, kind="ExternalInput")
    out = nc.dram_tensor("out", [N2, C], F32, kind="ExternalOutput")
    # DRAM scratch for tiny cross-partition relayouts
    g_dram = nc.dram_tensor("g_scratch", [N1], F32, kind="Internal")
    cnt_dram = nc.dram_tensor("cnt_scratch", [N2], F32, kind="Internal")

    # x rows factored: row = 256*t + 2*p + o  (t tile, p partition, o parity)
    xv = x[:].rearrange("(t p o) c -> o t p c", t=NT, p=P, o=2)

    with tile.TileContext(nc) as tc:
        with (
            tc.tile_pool(name="const", bufs=1) as cpool,
            tc.tile_pool(name="big", bufs=1) as bpool,
            tc.tile_pool(name="small", bufs=1) as spool,
        ):
            # ---- constants ----
            ident = cpool.tile([P, P], F32)
            make_identity(nc, ident[:])
            iota2048 = cpool.tile([P, N1], F32)
            nc.gpsimd.iota(iota2048[:], pattern=[[1, N1]], base=0,
                           channel_multiplier=0,
                           allow_small_or_imprecise_dtypes=True)
            iota1024 = cpool.tile([P, N2], F32)
            nc.gpsimd.iota(iota1024[:], pattern=[[1, N2]], base=0,
                           channel_multiplier=0,
                           allow_small_or_imprecise_dtypes=True)
            ones_col = cpool.tile([P, 1], F32)
            nc.vector.memset(ones_col[:], 1.0)
            ones_col_bf = cpool.tile([P, 1], BF16)
            nc.vector.memset(ones_col_bf[:], 1.0)
            ones_row1 = cpool.tile([1, P], F32)
            nc.vector.memset(ones_row1[:], 1.0)

            # ---- persistent buffers ----
            mx_even = bpool.tile([P, N1], F32)      # [tok, d] tiles, even toks
            mxT_even = bpool.tile([P, N1], F32)     # [d, tok] raw
            mxT_odd = bpool.tile([P, N1], F32)      # [d, tok] raw
            mxT_odd_n = bpool.tile([P, N1], F32)    # [d, tok] normalized cols
            macc_eT = bpool.tile([P, N2], F32)      # [d, j2] raw (even2)
            macc_oT = bpool.tile([P, N2], F32)      # [d, j2] raw (odd2)
            macc_oT_n = bpool.tile([P, N2], F32)
            g_bcast = bpool.tile([P, N1], F32)
            idx_b1 = spool.tile([P, NT * 8], U32)   # max_index outs, step 1
            idx_b2 = spool.tile([P, 8 * 8], U32)    # step 2
            idx1f = spool.tile([P, NT], F32)
            F_all = spool.tile([P, 2 * NT], F32)    # cols 0..15 even, 16.. odd
            s2r = spool.tile([P, 8], F32)

            # ================= Phase A: head-sums + transposes =============
            with (
                tc.tile_pool(name="xa", bufs=3) as xa,
                tc.tile_pool(name="psA", bufs=2, space="PSUM") as psA,
            ):
                for o in range(2):
                    for ti in range(NT):
                        xt = xa.tile([P, C], F32, tag="xt", name=f"xt{o}_{ti}")
                        nc.sync.dma_start(xt[:], xv[o, ti])
                        # tree-reduce 32 head blocks of 128 -> head sum
                        w = C
                        while w > D:
                            h = w // 2
                            nc.vector.tensor_add(xt[:, :h], xt[:, :h],
                                                 xt[:, h:w])
                            w = h
                        if o == 0:
                            nc.scalar.copy(
                                mx_even[:, ti * D:(ti + 1) * D], xt[:, :D])
                        pt = psA.tile([P, P], F32, tag="tr", name=f"tr{o}_{ti}")
                        nc.tensor.transpose(pt[:], xt[:, :D], ident[:])
                        dst = mxT_even if o == 0 else mxT_odd
                        nc.scalar.copy(dst[:, ti * D:(ti + 1) * D], pt[:])
            if lvl == 0:
                nc.sync.dma_start(out[0:P, 0:N1], mx_even[:])
                nc.sync.dma_start(out[0:P, N1:2 * N1], mxT_even[:])
                nc.sync.dma_start(out[P:2 * P, 0:N1], mxT_odd[:])

            # ============ Phase B: normalize odd cols, scores1, argmax =====
            if lvl >= 1:
                with (
                    tc.tile_pool(name="nb", bufs=1) as nb,
                    tc.tile_pool(name="psN", bufs=1, space="PSUM") as psN,
                ):
                    # n2 row = ones^T @ (mxT_odd^2)
                    sq = nb.tile([P, N1], F32)
                    nc.scalar.activation(sq[:], mxT_odd[:],
                                         mybir.ActivationFunctionType.Square)
                    n2row = nb.tile([1, N1], F32)
                    for jc in range(4):
                        pn = psN.tile([1, 512], F32, tag="n2", name=f"n2_{jc}")
                        nc.tensor.matmul(pn[:], ones_col[:],
                                         sq[:, jc * 512:(jc + 1) * 512],
                                         start=True, stop=True)
                        nc.scalar.copy(n2row[:, jc * 512:(jc + 1) * 512],
                                       pn[:])
                    rinv = nb.tile([1, N1], F32)
                    sqr = nb.tile([1, N1], F32)
                    nc.scalar.activation(sqr[:], n2row[:],
                                         mybir.ActivationFunctionType.Sqrt)
                    nc.vector.reciprocal(rinv[:], sqr[:])
                    _newton_rsqrt(nc, nb, rinv[:], n2row[:], [1, N1])
                    # broadcast rinv across partitions via K=1 ones matmul
                    for jc in range(4):
                        pb = psN.tile([P, 512], F32, tag="bc", name=f"bc_{jc}")
                        nc.tensor.matmul(pb[:], ones_row1[:],
                                         rinv[:, jc * 512:(jc + 1) * 512],
                                         start=True, stop=True)
                        nc.vector.tensor_mul(
                            mxT_odd_n[:, jc * 512:(jc + 1) * 512],
                            mxT_odd[:, jc * 512:(jc + 1) * 512], pb[:])

                with (
                    tc.tile_pool(name="sc1", bufs=2) as sc1,
                    tc.tile_pool(name="ps1", bufs=2, space="PSUM") as ps1,
                ):
                    m8 = spool.tile([P, 8], F32)
                    for ti in range(NT):
                        pscore = ps1.tile([P, N1], F32, tag="sc",
                                          name=f"sc{ti}")
                        for jc in range(4):
                            nc.tensor.matmul(
                                pscore[:, jc * 512:(jc + 1) * 512],
                                mxT_even[:, ti * D:(ti + 1) * D],
                                mxT_odd_n[:, jc * 512:(jc + 1) * 512],
                                start=True, stop=True)
                        ssb = sc1.tile([P, N1], F32, tag="ssb",
                                       name=f"ssb{ti}")
                        nc.scalar.copy(ssb[:], pscore[:])
                        nc.vector.max(m8[:], ssb[:])
                        nc.vector.max_index(idx_b1[:, ti * 8:(ti + 1) * 8],
                                            m8[:], ssb[:])
                    nc.vector.tensor_copy(idx1f[:], idx_b1[:, ::8])
                if lvl == 1:
                    nc.sync.dma_start(out[0:P, 0:NT], idx1f[:])
                    nc.sync.dma_start(out[0:P, 32:32 + N1], mxT_odd_n[:])

            # ========= Phase C: macc via one-hot matmuls, scores2 ==========
            if lvl >= 2:
                with (
                    tc.tile_pool(name="s1p", bufs=2) as s1p,
                    tc.tile_pool(name="psM", bufs=1, space="PSUM") as psM,
                ):
                    pmacc = psM.tile([P, N1], F32, tag="macc")
                    for ti in range(NT):
                        s1t = s1p.tile([P, N1], F32, tag="s1", name=f"s1_{ti}")
                        nc.vector.tensor_single_scalar(
                            s1t[:], iota2048[:], idx1f[:, ti:ti + 1],
                            AL.is_equal)
                        for jc in range(4):
                            nc.tensor.matmul(
                                pmacc[:, jc * 512:(jc + 1) * 512],
                                mx_even[:, ti * D:(ti + 1) * D],
                                s1t[:, jc * 512:(jc + 1) * 512],
                                start=(ti == 0), stop=(ti == NT - 1),
                                skip_group_check=True)
                    # macc^T = mxT_odd_raw + pmacc, deinterleaved even2/odd2
                    nc.vector.tensor_add(macc_eT[:], pmacc[:, ::2],
                                         mxT_odd[:, ::2])
                    nc.vector.tensor_add(macc_oT[:], pmacc[:, 1::2],
                                         mxT_odd[:, 1::2])

                with (
                    tc.tile_pool(name="nb2", bufs=1) as nb2,
                    tc.tile_pool(name="psN2", bufs=1, space="PSUM") as psN2,
                ):
                    sq2 = nb2.tile([P, N2], F32)
                    nc.scalar.activation(sq2[:], macc_oT[:],
                                         mybir.ActivationFunctionType.Square)
                    n2row2 = nb2.tile([1, N2], F32)
                    for jc in range(2):
                        pn = psN2.tile([1, 512], F32, tag="n2b",
                                       name=f"n2b_{jc}")
                        nc.tensor.matmul(pn[:], ones_col[:],
                                         sq2[:, jc * 512:(jc + 1) * 512],
                                         start=True, stop=True)
                        nc.scalar.copy(n2row2[:, jc * 512:(jc + 1) * 512],
                                       pn[:])
                    rinv2 = nb2.tile([1, N2], F32)
                    sqr2 = nb2.tile([1, N2], F32)
                    nc.scalar.activation(sqr2[:], n2row2[:],
                                         mybir.ActivationFunctionType.Sqrt)
                    nc.vector.reciprocal(rinv2[:], sqr2[:])
                    _newton_rsqrt(nc, nb2, rinv2[:], n2row2[:], [1, N2])
                    for jc in range(2):
                        pb = psN2.tile([P, 512], F32, tag="bcb",
                                       name=f"bcb_{jc}")
                        nc.tensor.matmul(pb[:], ones_row1[:],
                                         rinv2[:, jc * 512:(jc + 1) * 512],
                                         start=True, stop=True)
                        nc.vector.tensor_mul(
                            macc_oT_n[:, jc * 512:(jc + 1) * 512],
                            macc_oT[:, jc * 512:(jc + 1) * 512], pb[:])

                with (
                    tc.tile_pool(name="sc2", bufs=2) as sc2,
                    tc.tile_pool(name="ps2", bufs=2, space="PSUM") as ps2,
                ):
                    m8b = spool.tile([P, 8], F32)
                    for t2 in range(8):
                        pscore2 = ps2.tile([P, N2], F32, tag="sc2",
                                           name=f"sc2_{t2}")
                        for jc in range(2):
                            nc.tensor.matmul(
                                pscore2[:, jc * 512:(jc + 1) * 512],
                                macc_eT[:, t2 * D:(t2 + 1) * D],
                                macc_oT_n[:, jc * 512:(jc + 1) * 512],
                                start=True, stop=True)
                        ssb2 = sc2.tile([P, N2], F32, tag="ssb2",
                                        name=f"ssb2_{t2}")
                        nc.scalar.copy(ssb2[:], pscore2[:])
                        nc.vector.max(m8b[:], ssb2[:])
                        nc.vector.max_index(idx_b2[:, t2 * 8:(t2 + 1) * 8],
                                            m8b[:], ssb2[:])
                if lvl == 2:
                    idx2dbg = spool.tile([P, 8], F32)
                    nc.vector.tensor_copy(idx2dbg[:], idx_b2[:, ::8])
                    nc.sync.dma_start(out[0:P, 0:8], idx2dbg[:])
                    nc.sync.dma_start(out[0:P, 8:8 + N2], macc_eT[:])
                    nc.sync.dma_start(out[P:2 * P, 0:N2], macc_oT[:])

            # ================= Phase D: compose F ==========================
            if lvl >= 3:
                with (
                    tc.tile_pool(name="cmp", bufs=1) as cmp,
                    tc.tile_pool(name="s1d", bufs=2) as s1d,
                    tc.tile_pool(name="psD", bufs=1, space="PSUM") as psD,
                ):
                    idx2f = cmp.tile([P, 8], F32)
                    nc.vector.tensor_copy(idx2f[:], idx_b2[:, ::8])
                    # g row [1, 2048]: even o -> idx2[o/2], odd o -> (o-1)/2
                    g_row = cmp.tile([1, N1], F32)
                    nc.gpsimd.iota(g_row[0:1, 1::2], pattern=[[1, N2]], base=0,
                                   channel_multiplier=0,
                                   allow_small_or_imprecise_dtypes=True)
                    # even positions via DRAM bounce of idx2f:
                    #   g_dram[2*k], k = 128*t2 + p  ->  offset 256*t2 + 2*p
                    gv = g_dram[:].rearrange("(t p o) -> o p t", t=8, p=P, o=2)
                    nc.sync.dma_start(gv[0], idx2f[:])
                    gk = g_dram[:].rearrange("(k o) -> o k", o=2)
                    nc.sync.dma_start(g_row[0:1, 0::2], gk[0][None, :])
                    # F_odd[p, ti] = g[128*ti + p]: write full g, reload
                    nc.sync.dma_start(g_dram[:][None, :], g_row[:])
                    gf = g_dram[:].rearrange("(t p) -> p t", t=NT, p=P)
                    nc.sync.dma_start(F_all[:, NT:2 * NT], gf)
                    # g broadcast [128, 2048] via K=1 matmul
                    for jc in range(4):
                        pb = psD.tile([P, 512], F32, tag="gb", name=f"gb{jc}")
                        nc.tensor.matmul(pb[:], ones_row1[:],
                                         g_row[:, jc * 512:(jc + 1) * 512],
                                         start=True, stop=True)
                        nc.scalar.copy(g_bcast[:, jc * 512:(jc + 1) * 512],
                                       pb[:])
                    # F_even[p, ti] = sum_j S1[ti][p, j] * g[j]
                    ttr_scratch = cmp.tile([P, N1], F32)
                    for ti in range(NT):
                        s1t = s1d.tile([P, N1], F32, tag="s1d",
                                       name=f"s1d_{ti}")
                        nc.vector.tensor_single_scalar(
                            s1t[:], iota2048[:], idx1f[:, ti:ti + 1],
                            AL.is_equal)
                        nc.vector.tensor_mul(ttr_scratch[:], s1t[:],
                                             g_bcast[:])
                        nc.vector.tensor_reduce(
                            F_all[:, ti:ti + 1], ttr_scratch[:],
                            mybir.AxisListType.X, AL.add)
                if lvl == 3:
                    nc.sync.dma_start(out[0:P, 0:2 * NT], F_all[:])

            # ================= Phase E: final scatter ======================
            if lvl >= 4:
                with (
                    tc.tile_pool(name="sf", bufs=1) as sfp,
                    tc.tile_pool(name="cntp", bufs=1) as cntp,
                    tc.tile_pool(name="xe", bufs=4) as xe,
                    tc.tile_pool(name="oe", bufs=3) as oe,
                ):
                    sf_tiles = []
                    for t in range(2 * NT):
                        sft = sfp.tile([P, N2], BF16, tag=f"sf{t}",
                                       name=f"sf_{t}")
                        nc.vector.tensor_single_scalar(
                            sft[:], iota1024[:], F_all[:, t:t + 1],
                            AL.is_equal)
                        sf_tiles.append(sft)
                    # counts: cnt[1, 1024] = ones^T @ S_F (bf16, exact ints)
                    with tc.tile_pool(name="psC", bufs=1,
                                      space="PSUM") as psC:
                        cnt_row = cntp.tile([1, N2], F32)
                        for jc in range(2):
                            pc = psC.tile([1, 512], F32, tag="cnt",
                                          name=f"cnt{jc}")
                            for t in range(2 * NT):
                                nc.tensor.matmul(
                                    pc[:], ones_col_bf[:],
                                    sf_tiles[t][:, jc * 512:(jc + 1) * 512],
                                    start=(t == 0), stop=(t == 2 * NT - 1),
                                    skip_group_check=True)
                            nc.scalar.copy(
                                cnt_row[:, jc * 512:(jc + 1) * 512], pc[:])
                    # counts -> [128, 8] via DRAM bounce, reciprocal
                    nc.sync.dma_start(cnt_dram[:][None, :], cnt_row[:])
                    cnt_col = cntp.tile([P, 8], F32)
                    cv = cnt_dram[:].rearrange("(b p) -> p b", b=8, p=P)
                    nc.sync.dma_start(cnt_col[:], cv)
                    nc.vector.reciprocal(s2r[:], cnt_col[:])

                    with tc.tile_pool(name="psE", bufs=1,
                                      space="PSUM") as psE:
                        for cb in range(8):
                            accs = [psE.tile([P, 512], F32, tag=f"acc{b}",
                                             name=f"acc_{cb}_{b}")
                                    for b in range(8)]
                            for t in range(2 * NT):
                                o, ti = t // NT, t % NT
                                xt = xe.tile([P, 512], BF16, tag="xt",
                                             name=f"xe_{cb}_{t}")
                                nc.gpsimd.dma_start(
                                    xt[:],
                                    xv[o, ti, :, cb * 512:(cb + 1) * 512])
                                for b in range(8):
                                    nc.tensor.matmul(
                                        accs[b][:],
                                        sf_tiles[t][:, b * P:(b + 1) * P],
                                        xt[:],
                                        start=(t == 0),
                                        stop=(t == 2 * NT - 1),
                                        skip_group_check=True)
                            for b in range(8):
                                osb = oe.tile([P, 512], F32, tag="osb",
                                              name=f"osb_{cb}_{b}")
                                nc.vector.tensor_scalar_mul(
                                    osb[:], accs[b][:], s2r[:, b:b + 1])
                                nc.sync.dma_start(
                                    out[b * P:(b + 1) * P,
                                        cb * 512:(cb + 1) * 512],
                                    osb[:])

    nc.finalize()
    return nc


_CACHED = None


def kernel(x: np.ndarray, target_num_token=None) -> np.ndarray:
    """Full-input entry point: x [8, 4096, 4096] fp32 -> [8, 1024, 4096]."""
    global _CACHED
    x = np.ascontiguousarray(np.asarray(x), dtype=np.float32)
    b = x.shape[0]
    assert x.shape == (8, PTOK, C), x.shape
    if _CACHED is None:
        _CACHED = build_kernel()
    nc = _CACHED
    in_maps = [{"x": x[i]} for i in range(b)]
    res = run_bass_kernel_spmd(nc, in_maps, core_ids=list(range(b)))
    return np.stack([res.results[i]["out"] for i in range(b)])


if __name__ == "__main__":
    x = np.load("/root/problem/x_input.npy")
    y = kernel(x)
    print("kernel output", y.shape, y.dtype)
    np.save("/root/problem/y_kernel.npy", y)


# revision 14
# speedup vs baseline: 1.0139x; 1.0139x over previous
"""Trainium2 Bass kernel for nn_MemoryManager (ToMe token merging).

Problem: x [8, 4096, 4096] fp32, target 1024 tokens. The reference performs 2
bipartite merge steps (4096->2048->1024). Because r == p/2 at both steps, the
"unmerged" set is empty and the argsort is dead code; the weighted sums
telescope so the final output is a single scatter-add of all 4096 original
rows into 1024 accumulators (divided by multiplicity), with all control logic
(metric means, cosine scores, argmax, index composition) in 128-dim metric
space.

Sharding: data-parallel over batch (8 cores, 1 batch element each).

Per-core pipeline:
  A) stream x tiles (even/odd token split), head-sum via tree adds -> mx
     [tok,d]; PE-transpose -> mxT [d,tok].
  B) normalize odd-side metric columns (ones-matmul norms + exact reciprocal
     of sqrt + 2 Newton steps); scores1 = mxT_even^T @ mxT_odd_n in fp32;
     hardware argmax (max/max_index) -> idx1.
  C) one-hot(idx1) matmuls accumulate macc^T = mxT_odd_raw + scatter(mx_even);
     normalize odd columns; scores2 -> idx2.
  D) compose final assignment F (token -> output row) via g-table reduce.
  E) scatter: one-hot(F) matmuls in bf16 (one-hot is exact; x cast to bf16
     during DMA) accumulate out = S_F^T @ x per (dst-block, c-block) in PSUM;
     counts via ones^T @ S_F; out = acc * (1/s2).
"""

import numpy as np

import concourse.bacc as bacc
import concourse.bass as bass
import concourse.mybir as mybir
import concourse.tile as tile
from concourse.bass_utils import run_bass_kernel_spmd
from concourse.masks import make_identity

F32 = mybir.dt.float32
BF16 = mybir.dt.bfloat16
U32 = mybir.dt.uint32

P = 128           # partitions
NT = 16           # token tiles per parity (16 * 128 even + 16 * 128 odd = 4096)
C = 4096          # channels
D = 128           # metric dim (C / 32 heads)
PTOK = 4096       # tokens
N1 = 2048         # step-1 odd/even count
N2 = 1024         # step-2 odd/even count = output rows
AL = mybir.AluOpType

_LVL = {"A": 0, "B": 1, "C": 2, "D": 3, "E": 4}


def _newton_rsqrt(nc, pool, y, n2, shape):
    """y <- rsqrt(n2) refined: two Newton steps y *= 1.5 - 0.5*n2*y*y."""
    t = pool.tile(shape, F32, tag="newt_t", name="newt_t")
    for _ in range(2):
        nc.vector.tensor_mul(t[:], y, y)
        nc.vector.tensor_mul(t[:], t[:], n2)
        nc.vector.tensor_scalar(t[:], t[:], -0.5, 1.5, AL.mult, AL.add)
        nc.vector.tensor_mul(y, y, t[:])


def build_kernel(stop_after="E"):
    lvl = _LVL[stop_after]
    nc = bacc.Bacc(None, target_bir_lowering=False)
    x = nc.dram_tensor("x", [PTOK, C], F32, kind="ExternalInput")
    out = nc.dram_tensor("out", [N2, C], F32, kind="ExternalOutput")
    # DRAM scratch for tiny cross-partition relayouts
    g_dram = nc.dram_tensor("g_scratch", [N1], F32, kind="Internal")
    cnt_dram = nc.dram_tensor("cnt_scratch", [N2], F32, kind="Internal")
    i1_dram = nc.dram_tensor("i1_scratch", [N1], F32, kind="Internal")

    # x rows factored: row = 256*t + 2*p + o  (t tile, p partition, o parity)
    xv = x[:].rearrange("(t p o) c -> o t p c", t=NT, p=P, o=2)

    with tile.TileContext(nc) as tc:
        with (
            tc.tile_pool(name="const", bufs=1) as cpool,
            tc.tile_pool(name="big", bufs=1) as bpool,
            tc.tile_pool(name="small", bufs=1) as spool,
        ):
            # ---- constants ----
            ident = cpool.tile([P, P], F32)
            make_identity(nc, ident[:])
            iota2048 = cpool.tile([P, N1], F32)
            nc.gpsimd.iota(iota2048[:], pattern=[[1, N1]], base=0,
                           channel_multiplier=0,
                           allow_small_or_imprecise_dtypes=True)
            iota1024 = cpool.tile([P, N2], F32)
            nc.gpsimd.iota(iota1024[:], pattern=[[1, N2]], base=0,
                           channel_multiplier=0,
                           allow_small_or_imprecise_dtypes=True)
            ones_col = cpool.tile([P, 1], F32)
            nc.vector.memset(ones_col[:], 1.0)
            ones_col_bf = cpool.tile([P, 1], BF16)
            nc.vector.memset(ones_col_bf[:], 1.0)
            ones_row1 = cpool.tile([1, P], F32)
            nc.vector.memset(ones_row1[:], 1.0)
            # iota_pcol[p, jt] = p + 128*jt (j-value of partition p in j-tile jt)
            iota_pcol = cpool.tile([P, NT], F32)
            nc.gpsimd.iota(iota_pcol[:], pattern=[[P, NT]], base=0,
                           channel_multiplier=1,
                           allow_small_or_imprecise_dtypes=True)

            # ---- persistent buffers ----
            mx_even = bpool.tile([P, N1], F32)      # [tok, d] tiles, even toks
            mxT_even = bpool.tile([P, N1], F32)     # [d, tok] raw
            mxT_odd = bpool.tile([P, N1], F32)      # [d, tok] raw
            mxT_odd_n = bpool.tile([P, N1], F32)    # [d, tok] normalized cols
            macc_eT = bpool.tile([P, N2], F32)      # [d, j2] raw (even2)
            macc_oT = bpool.tile([P, N2], F32)      # [d, j2] raw (odd2)
            macc_oT_n = bpool.tile([P, N2], F32)
            g_bcast = bpool.tile([P, N1], F32)
            idx_b1 = spool.tile([P, NT * 8], U32)   # max_index outs, step 1
            idx_b2 = spool.tile([P, 8 * 8], U32)    # step 2
            idx1f = spool.tile([P, NT], F32)
            F_all = spool.tile([P, 2 * NT], F32)    # cols 0..15 even, 16.. odd
            s2r = spool.tile([P, 8], F32)

            # ================= Phase A: head-sums + transposes =============
            with (
                tc.tile_pool(name="xa", bufs=3) as xa,
                tc.tile_pool(name="psA", bufs=2, space="PSUM") as psA,
            ):
                for o in range(2):
                    for ti in range(NT):
                        xt = xa.tile([P, C], F32, tag="xt", name=f"xt{o}_{ti}")
                        nc.sync.dma_start(xt[:], xv[o, ti])
                        # tree-reduce 32 head blocks of 128 -> head sum
                        w = C
                        while w > D:
                            h = w // 2
                            nc.vector.tensor_add(xt[:, :h], xt[:, :h],
                                                 xt[:, h:w])
                            w = h
                        if o == 0:
                            nc.scalar.copy(
                                mx_even[:, ti * D:(ti + 1) * D], xt[:, :D])
                        pt = psA.tile([P, P], F32, tag="tr", name=f"tr{o}_{ti}")
                        nc.tensor.transpose(pt[:], xt[:, :D], ident[:])
                        dst = mxT_even if o == 0 else mxT_odd
                        nc.scalar.copy(dst[:, ti * D:(ti + 1) * D], pt[:])
            if lvl == 0:
                nc.sync.dma_start(out[0:P, 0:N1], mx_even[:])
                nc.sync.dma_start(out[0:P, N1:2 * N1], mxT_even[:])
                nc.sync.dma_start(out[P:2 * P, 0:N1], mxT_odd[:])

            # ============ Phase B: normalize odd cols, scores1, argmax =====
            if lvl >= 1:
                with (
                    tc.tile_pool(name="nb", bufs=1) as nb,
                    tc.tile_pool(name="psN", bufs=1, space="PSUM") as psN,
                ):
                    # n2 row = ones^T @ (mxT_odd^2)
                    sq = nb.tile([P, N1], F32)
                    nc.scalar.activation(sq[:], mxT_odd[:],
                                         mybir.ActivationFunctionType.Square)
                    n2row = nb.tile([1, N1], F32)
                    for jc in range(4):
                        pn = psN.tile([1, 512], F32, tag="n2", name=f"n2_{jc}")
                        nc.tensor.matmul(pn[:], ones_col[:],
                                         sq[:, jc * 512:(jc + 1) * 512],
                                         start=True, stop=True)
                        nc.scalar.copy(n2row[:, jc * 512:(jc + 1) * 512],
                                       pn[:])
                    rinv = nb.tile([1, N1], F32)
                    sqr = nb.tile([1, N1], F32)
                    nc.scalar.activation(sqr[:], n2row[:],
                                         mybir.ActivationFunctionType.Sqrt)
                    nc.vector.reciprocal(rinv[:], sqr[:])
                    _newton_rsqrt(nc, nb, rinv[:], n2row[:], [1, N1])
                    # broadcast rinv across partitions via K=1 ones matmul
                    for jc in range(4):
                        pb = psN.tile([P, 512], F32, tag="bc", name=f"bc_{jc}")
                        nc.tensor.matmul(pb[:], ones_row1[:],
                                         rinv[:, jc * 512:(jc + 1) * 512],
                                         start=True, stop=True)
                        nc.vector.tensor_mul(
                            mxT_odd_n[:, jc * 512:(jc + 1) * 512],
                            mxT_odd[:, jc * 512:(jc + 1) * 512], pb[:])

                with (
                    tc.tile_pool(name="sc1", bufs=2) as sc1,
                    tc.tile_pool(name="ps1", bufs=2, space="PSUM") as ps1,
                ):
                    m8 = spool.tile([P, 8], F32)
                    for ti in range(NT):
                        pscore = ps1.tile([P, N1], F32, tag="sc",
                                          name=f"sc{ti}")
                        for jc in range(4):
                            nc.tensor.matmul(
                                pscore[:, jc * 512:(jc + 1) * 512],
                                mxT_even[:, ti * D:(ti + 1) * D],
                                mxT_odd_n[:, jc * 512:(jc + 1) * 512],
                                start=True, stop=True)
                        ssb = sc1.tile([P, N1], F32, tag="ssb",
                                       name=f"ssb{ti}")
                        nc.scalar.copy(ssb[:], pscore[:])
                        nc.vector.max(m8[:], ssb[:])
                        nc.vector.max_index(idx_b1[:, ti * 8:(ti + 1) * 8],
                                            m8[:], ssb[:])
                    nc.vector.tensor_copy(idx1f[:], idx_b1[:, ::8])
                if lvl == 1:
                    nc.sync.dma_start(out[0:P, 0:NT], idx1f[:])
                    nc.sync.dma_start(out[0:P, 32:32 + N1], mxT_odd_n[:])

            # ========= Phase C: macc via one-hot matmuls, scores2 ==========
            if lvl >= 2:
                with (
                    tc.tile_pool(name="s1p", bufs=2) as s1p,
                    tc.tile_pool(name="psM", bufs=1, space="PSUM") as psM,
                ):
                    pmacc = psM.tile([P, N1], F32, tag="macc")
                    for ti in range(NT):
                        s1t = s1p.tile([P, N1], F32, tag="s1", name=f"s1_{ti}")
                        nc.vector.tensor_single_scalar(
                            s1t[:], iota2048[:], idx1f[:, ti:ti + 1],
                            AL.is_equal)
                        for jc in range(4):
                            nc.tensor.matmul(
                                pmacc[:, jc * 512:(jc + 1) * 512],
                                mx_even[:, ti * D:(ti + 1) * D],
                                s1t[:, jc * 512:(jc + 1) * 512],
                                start=(ti == 0), stop=(ti == NT - 1),
                                skip_group_check=True)
                    # macc^T = mxT_odd_raw + pmacc, deinterleaved even2/odd2
                    nc.vector.tensor_add(macc_eT[:], pmacc[:, ::2],
                                         mxT_odd[:, ::2])
                    nc.vector.tensor_add(macc_oT[:], pmacc[:, 1::2],
                                         mxT_odd[:, 1::2])

                with (
                    tc.tile_pool(name="nb2", bufs=1) as nb2,
                    tc.tile_pool(name="psN2", bufs=1, space="PSUM") as psN2,
                ):
                    sq2 = nb2.tile([P, N2], F32)
                    nc.scalar.activation(sq2[:], macc_oT[:],
                                         mybir.ActivationFunctionType.Square)
                    n2row2 = nb2.tile([1, N2], F32)
                    for jc in range(2):
                        pn = psN2.tile([1, 512], F32, tag="n2b",
                                       name=f"n2b_{jc}")
                        nc.tensor.matmul(pn[:], ones_col[:],
                                         sq2[:, jc * 512:(jc + 1) * 512],
                                         start=True, stop=True)
                        nc.scalar.copy(n2row2[:, jc * 512:(jc + 1) * 512],
                                       pn[:])
                    rinv2 = nb2.tile([1, N2], F32)
                    sqr2 = nb2.tile([1, N2], F32)
                    nc.scalar.activation(sqr2[:], n2row2[:],
                                         mybir.ActivationFunctionType.Sqrt)
                    nc.vector.reciprocal(rinv2[:], sqr2[:])
                    _newton_rsqrt(nc, nb2, rinv2[:], n2row2[:], [1, N2])
                    for jc in range(2):
                        pb = psN2.tile([P, 512], F32, tag="bcb",
                                       name=f"bcb_{jc}")
                        nc.tensor.matmul(pb[:], ones_row1[:],
                                         rinv2[:, jc * 512:(jc + 1) * 512],
                                         start=True, stop=True)
                        nc.vector.tensor_mul(
                            macc_oT_n[:, jc * 512:(jc + 1) * 512],
                            macc_oT[:, jc * 512:(jc + 1) * 512], pb[:])

                with (
                    tc.tile_pool(name="sc2", bufs=2) as sc2,
                    tc.tile_pool(name="ps2", bufs=2, space="PSUM") as ps2,
                ):
                    m8b = spool.tile([P, 8], F32)
                    for t2 in range(8):
                        pscore2 = ps2.tile([P, N2], F32, tag="sc2",
                                           name=f"sc2_{t2}")
                        for jc in range(2):
                            nc.tensor.matmul(
                                pscore2[:, jc * 512:(jc + 1) * 512],
                                macc_eT[:, t2 * D:(t2 + 1) * D],
                                macc_oT_n[:, jc * 512:(jc + 1) * 512],
                                start=True, stop=True)
                        ssb2 = sc2.tile([P, N2], F32, tag="ssb2",
                                        name=f"ssb2_{t2}")
                        nc.scalar.copy(ssb2[:], pscore2[:])
                        nc.vector.max(m8b[:], ssb2[:])
                        nc.vector.max_index(idx_b2[:, t2 * 8:(t2 + 1) * 8],
                                            m8b[:], ssb2[:])
                if lvl == 2:
                    idx2dbg = spool.tile([P, 8], F32)
                    nc.vector.tensor_copy(idx2dbg[:], idx_b2[:, ::8])
                    nc.sync.dma_start(out[0:P, 0:8], idx2dbg[:])
                    nc.sync.dma_start(out[0:P, 8:8 + N2], macc_eT[:])
                    nc.sync.dma_start(out[P:2 * P, 0:N2], macc_oT[:])

            # ================= Phase D: compose F ==========================
            if lvl >= 3:
                with (
                    tc.tile_pool(name="cmp", bufs=1) as cmp,
                    tc.tile_pool(name="s1d", bufs=2) as s1d,
                    tc.tile_pool(name="psD", bufs=1, space="PSUM") as psD,
                ):
                    idx2f = cmp.tile([P, 8], F32)
                    nc.vector.tensor_copy(idx2f[:], idx_b2[:, ::8])
                    # g row [1, 2048]: even o -> idx2[o/2], odd o -> (o-1)/2
                    g_row = cmp.tile([1, N1], F32)
                    nc.gpsimd.iota(g_row[0:1, 1::2], pattern=[[1, N2]], base=0,
                                   channel_multiplier=0,
                                   allow_small_or_imprecise_dtypes=True)
                    # even positions via DRAM bounce of idx2f:
                    #   g_dram[2*k], k = 128*t2 + p  ->  offset 256*t2 + 2*p
                    gv = g_dram[:].rearrange("(t p o) -> o p t", t=8, p=P, o=2)
                    nc.sync.dma_start(gv[0], idx2f[:])
                    gk = g_dram[:].rearrange("(k o) -> o k", o=2)
                    nc.sync.dma_start(g_row[0:1, 0::2], gk[0][None, :])
                    # F_odd[p, ti] = g[128*ti + p]: write full g, reload.
                    # (This column layout of g is also the matmul operand
                    # below.)
                    nc.sync.dma_start(g_dram[:][None, :], g_row[:])
                    gf = g_dram[:].rearrange("(t p) -> p t", t=NT, p=P)
                    nc.sync.dma_start(F_all[:, NT:2 * NT], gf)
                    # idx1 as a broadcast row [128, 2048] via DRAM bounce +
                    # K=1 ones matmul: idx1_bc[p, i] = idx1[i], i = 128*ti + p
                    i1d = i1_dram[:].rearrange("(t p) -> p t", t=NT, p=P)
                    nc.sync.dma_start(i1d, idx1f[:])
                    i1row = cmp.tile([1, N1], F32)
                    nc.sync.dma_start(i1row[:], i1_dram[:][None, :])
                    idx1_bc = cmp.tile([P, N1], F32)
                    for jc in range(4):
                        pb = psD.tile([P, 512], F32, tag="gb", name=f"gb{jc}")
                        nc.tensor.matmul(pb[:], ones_row1[:],
                                         i1row[:, jc * 512:(jc + 1) * 512],
                                         start=True, stop=True)
                        nc.scalar.copy(idx1_bc[:, jc * 512:(jc + 1) * 512],
                                       pb[:])
                    # F_even: F_even[i] = g[idx1[i]] = sum_j S1T[j, i]*g[j]
                    # S1T[j-tile jt][p_j, i] = (idx1_bc[p_j, i] == p_j+128*jt)
                    # F_row [1, 2048] += g_col_jt^T @ S1T_jt, one accumulation
                    # group per 512-chunk bank (fp16 operands: ints <= 2047
                    # exact; PSUM accumulate fp32).
                    F16 = mybir.dt.float16
                    g16 = cmp.tile([P, NT], F16)
                    nc.vector.tensor_copy(g16[:], F_all[:, NT:2 * NT])
                    pfr = [psD.tile([1, 512], F32, tag=f"pfr{c}",
                                    name=f"pfr{c}") for c in range(4)]
                    for jt in range(NT):
                        s1tt = s1d.tile([P, N1], F16, tag="s1d",
                                        name=f"s1tt_{jt}")
                        nc.vector.tensor_single_scalar(
                            s1tt[:], idx1_bc[:], iota_pcol[:, jt:jt + 1],
                            AL.is_equal)
                        for ic in range(4):
                            nc.tensor.matmul(
                                pfr[ic][:],
                                g16[:, jt:jt + 1],
                                s1tt[:, ic * 512:(ic + 1) * 512],
                                start=(jt == 0), stop=(jt == NT - 1),
                                skip_group_check=True)
                    fe_row = cmp.tile([1, N1], F32)
                    for ic in range(4):
                        nc.scalar.copy(fe_row[:, ic * 512:(ic + 1) * 512],
                                       pfr[ic][:])
                    # bounce row -> F_all[:, 0:NT] column layout
                    nc.sync.dma_start(i1_dram[:][None, :], fe_row[:])
                    nc.sync.dma_start(
                        F_all[:, 0:NT],
                        i1_dram[:].rearrange("(t p) -> p t", t=NT, p=P))
                if lvl == 3:
                    nc.sync.dma_start(out[0:P, 0:2 * NT], F_all[:])

            # ================= Phase E: final scatter ======================
            if lvl >= 4:
                with (
                    tc.tile_pool(name="sf", bufs=1) as sfp,
                    tc.tile_pool(name="cntp", bufs=1) as cntp,
                    tc.tile_pool(name="xe", bufs=4) as xe,
                    tc.tile_pool(name="oe", bufs=3) as oe,
                ):
                    sf_tiles = []
                    for t in range(2 * NT):
                        sft = sfp.tile([P, N2], BF16, tag=f"sf{t}",
                                       name=f"sf_{t}")
                        nc.vector.tensor_single_scalar(
                            sft[:], iota1024[:], F_all[:, t:t + 1],
                            AL.is_equal)
                        sf_tiles.append(sft)
                    # counts: cnt[1, 1024] = ones^T @ S_F (bf16, exact ints)
                    with tc.tile_pool(name="psC", bufs=1,
                                      space="PSUM") as psC:
                        cnt_row = cntp.tile([1, N2], F32)
                        for jc in range(2):
                            pc = psC.tile([1, 512], F32, tag="cnt",
                                          name=f"cnt{jc}")
                            for t in range(2 * NT):
                                nc.tensor.matmul(
                                    pc[:], ones_col_bf[:],
                                    sf_tiles[t][:, jc * 512:(jc + 1) * 512],
                                    start=(t == 0), stop=(t == 2 * NT - 1),
                                    skip_group_check=True)
                            nc.scalar.copy(
                                cnt_row[:, jc * 512:(jc + 1) * 512], pc[:])
                    # counts -> [128, 8] via DRAM bounce, reciprocal
                    nc.sync.dma_start(cnt_dram[:][None, :], cnt_row[:])
                    cnt_col = cntp.tile([P, 8], F32)
                    cv = cnt_dram[:].rearrange("(b p) -> p b", b=8, p=P)
                    nc.sync.dma_start(cnt_col[:], cv)
                    nc.vector.reciprocal(s2r[:], cnt_col[:])

                    with tc.tile_pool(name="psE", bufs=1,
                                      space="PSUM") as psE:
                        for cb in range(8):
                            accs = [psE.tile([P, 512], F32, tag=f"acc{b}",
                                             name=f"acc_{cb}_{b}")
                                    for b in range(8)]
                            for t in range(2 * NT):
                                o, ti = t // NT, t % NT
                                xt = xe.tile([P, 512], BF16, tag="xt",
                                             name=f"xe_{cb}_{t}")
                                nc.gpsimd.dma_start(
                                    xt[:],
                                    xv[o, ti, :, cb * 512:(cb + 1) * 512])
                                for b in range(8):
                                    nc.tensor.matmul(
                                        accs[b][:],
                                        sf_tiles[t][:, b * P:(b + 1) * P],
                                        xt[:],
                                        start=(t == 0),
                                        stop=(t == 2 * NT - 1),
                                        skip_group_check=True)
                            for b in range(8):
                                osb = oe.tile([P, 512], F32, tag="osb",
                                              name=f"osb_{cb}_{b}")
                                nc.vector.tensor_scalar_mul(
                                    osb[:], accs[b][:], s2r[:, b:b + 1])
                                nc.sync.dma_start(
                                    out[b * P:(b + 1) * P,
                                        cb * 512:(cb + 1) * 512],
                                    osb[:])

    nc.finalize()
    return nc


_CACHED = None


def kernel(x: np.ndarray, target_num_token=None) -> np.ndarray:
    """Full-input entry point: x [8, 4096, 4096] fp32 -> [8, 1024, 4096]."""
    global _CACHED
    x = np.ascontiguousarray(np.asarray(x), dtype=np.float32)
    b = x.shape[0]
    assert x.shape == (8, PTOK, C), x.shape
    if _CACHED is None:
        _CACHED = build_kernel()
    nc = _CACHED
    in_maps = [{"x": x[i]} for i in range(b)]
    res = run_bass_kernel_spmd(nc, in_maps, core_ids=list(range(b)))
    return np.stack([res.results[i]["out"] for i in range(b)])


if __name__ == "__main__":
    x = np.load("/root/problem/x_input.npy")
    y = kernel(x)
    print("kernel output", y.shape, y.dtype)
    np.save("/root/problem/y_kernel.npy", y)
